# revision 3
# baseline (speedup 1.0000x reference)
"""DeepseekV3 MoE gate kernel for 8 TRN2 NeuronCores — v2.

Strategy: shard tokens 8192 -> 8 x 1024, replicate gate weight/bias.

GEMM (per core, logits^T[e,t] accumulated in PSUM at 2^16 scale):
  - pass 1 (fp16): w16s = fp16(w) * 2^16 stationary, x16 = fp16(x) moving.
  - pass 2 (fp8 DoubleRow): acc += w8^T xl8 + wl8^T x8 (both fp16-residual
    correction terms in one DR matmul per (k, e-tile, chunk)).
  v2 changes vs the 173.7us baseline:
  - x8 = e4m3(x16) is no longer shipped from host: the otherwise-idle ACT
    engine derives it on-chip (Copy is a filler function present in every
    ACT table set, so it never evicts the Sigmoid table). DMA drops from
    36.7 -> 29.4 MB/core.
  - k-outer / chunk-inner matmul loops: each stationary weight tile (fp16
    and the expensive 256-col DoubleRow pair) is loaded once and serves
    both 512-token chunks back-to-back, hiding LDWEIGHTS under the longer
    matmul stream.
  - epilogue: all 4 sigmoids run first so the PSUM accumulators free
    early (pacc bufs=1 + pot bufs=4 exactly fills the 8 PSUM banks); ACT
    copies each transposed score tile PSUM->SBUF so the PE transpose
    buffers recycle fast; the top-k chain then runs DVE-only as before.
"""
import contextlib
import sys

for _p in ("/opt/trn_rl_repo", "/opt/pypackages"):
    if _p not in sys.path:
        sys.path.append(_p)

import numpy as np
import concourse.bacc as bacc
import concourse.mybir as mybir
import concourse.tile as tile
from concourse import bass_utils

F32 = mybir.dt.float32
F16 = mybir.dt.float16
F8 = mybir.dt.float8e4
DR = mybir.MatmulPerfMode.DoubleRow
AF = mybir.ActivationFunctionType
OP = mybir.AluOpType
AX = mybir.AxisListType

TOKENS, HIDDEN, E = 8192, 7168, 256
N_CORES = 8
T = TOKENS // N_CORES          # 1024 tokens per core
KT = HIDDEN // 128             # 56 k-tiles
CH = 2                         # chunks (PSUM bank limit: 512 fp32/matmul)
CHT = T // CH                  # 512 tokens per chunk
GK = 8                         # k-tiles per streamed group
NG = KT // GK                  # 7 groups
SCALE = 2.0 ** 16              # pass-1/2 common scale
S_XL = 2.0 ** 13               # xl8 = e4m3(xl * S_XL); w8 = e4m3(w * 8)
ROUTED_SCALING = 2.5

_CACHE = {}


def _declare_io(nc):
    x16_d = nc.dram_tensor("x16", [128, KT, T], F16, kind="ExternalInput").ap()
    xl8_d = nc.dram_tensor("xl8", [128, KT, T], F8, kind="ExternalInput").ap()
    w16_d = nc.dram_tensor("w16s", [128, KT, E], F16, kind="ExternalInput").ap()
    wl8_d = nc.dram_tensor("wl8", [128, KT, E], F8, kind="ExternalInput").ap()
    w8_d = nc.dram_tensor("w8", [128, KT, E], F8, kind="ExternalInput").ap()
    biasp_d = nc.dram_tensor("bias_plain", [E], F32, kind="ExternalInput").ap()
    ident_d = nc.dram_tensor("ident", [128, 128], F32, kind="ExternalInput").ap()
    out_d = nc.dram_tensor("out", [T, E], F16, kind="ExternalOutput").ap()
    return x16_d, xl8_d, w16_d, wl8_d, w8_d, biasp_d, ident_d, out_d


def _make_pools(tc, ctx):
    return {
        "const": ctx.enter_context(tc.tile_pool(name="const", bufs=1)),
        "wt": ctx.enter_context(tc.tile_pool(name="wt", bufs=1)),
        "xf": ctx.enter_context(tc.tile_pool(name="xf", bufs=3)),
        "xq": ctx.enter_context(tc.tile_pool(name="xq", bufs=3)),
        "et": ctx.enter_context(tc.tile_pool(name="et", bufs=2)),
        "tk": ctx.enter_context(tc.tile_pool(name="tk", bufs=2)),
        "pacc": ctx.enter_context(tc.tile_pool(name="pacc", bufs=1, space="PSUM")),
        "pot": ctx.enter_context(tc.tile_pool(name="pot", bufs=4, space="PSUM")),
    }


def _body(nc, pools, x16_d, xl8_d, w16_d, wl8_d, w8_d, biasp_d, ident_d, out_d):
    const, xf, xq = pools["const"], pools["xf"], pools["xq"]
    wt = pools["wt"]
    et, tkp = pools["et"], pools["tk"]
    pacc, pot = pools["pacc"], pools["pot"]

    w16s_t = wt.tile([128, KT, E], F16, name="w16s_t", tag="w16s")
    w8p_t = wt.tile([128, 2, KT, E], F8, name="w8p_t", tag="w8p")
    bias_rep = const.tile([128, E], F32, name="bias_rep")
    ident = const.tile([128, 128], F32, name="ident")
    nc.sync.dma_start(ident, ident_d)

    def stream_weights(g):
        ks = slice(g * GK, (g + 1) * GK)
        nc.sync.dma_start(w16s_t[:, ks], w16_d[:, ks])
        nc.sync.dma_start(w8p_t[:, 1, ks], wl8_d[:, ks])
        nc.sync.dma_start(w8p_t[:, 0, ks], w8_d[:, ks])

    # logits^T accumulators: [e-tile][chunk], each one full PSUM bank
    accs = [[pacc.tile([128, CHT], F32, name=f"acc{e}{c}", tag=f"acc{e}{c}")
             for c in range(CH)] for e in (0, 1)]

    for g in range(NG):
        gsl = slice(g * GK, (g + 1) * GK)
        x16g = xf.tile([128, GK, T], F16, name=f"x16_{g}", tag="x16")
        src = x16_d[:, gsl]
        if g == 0:
            stream_weights(0)
            # halve the very first transfer so the first matmuls start
            # as soon as the leading k-tiles land
            nc.sync.dma_start(x16g[:, 0:GK // 2], src[:, 0:GK // 2])
            nc.sync.dma_start(x16g[:, GK // 2:], src[:, GK // 2:])
        else:
            nc.sync.dma_start(x16g, src)
        xqg = xq.tile([128, 2, GK, T], F8, name=f"xq_{g}", tag="xq")
        nc.sync.dma_start(xqg[:, 0], xl8_d[:, gsl])
        # derive x8 = e4m3(x16) on ACT (Copy lives in every table set)
        nc.scalar.copy(xqg[:, 1], x16g)
        if g < NG - 1:
            stream_weights(g + 1)
        if g == 0:
            nc.sync.dma_start(bias_rep,
                              biasp_d[None, :].to_broadcast([128, E]))
        # pass 1 (fp16): k-outer, chunk-inner (stationary tile shared)
        for ko in range(GK):
            k = g * GK + ko
            for e in (0, 1):
                for c in range(CH):
                    nc.tensor.matmul(
                        accs[e][c], w16s_t[:, k, e * 128:(e + 1) * 128],
                        x16g[:, ko, c * CHT:(c + 1) * CHT],
                        start=(k == 0), stop=False)
        # pass 2 (fp8 DoubleRow): same k-tiles, after the ACT cast landed
        for ko in range(GK):
            k = g * GK + ko
            last = (k == KT - 1)
            for e in (0, 1):
                for c in range(CH):
                    nc.tensor.matmul(
                        accs[e][c],
                        w8p_t[:, :, k, e * 128:(e + 1) * 128],
                        xqg[:, :, ko, c * CHT:(c + 1) * CHT],
                        start=False, stop=last, perf_mode=DR)

    # epilogue: sigmoids first (frees all 4 PSUM accumulator banks), then
    # per-128-token-group: PE transpose -> ACT copy to SBUF -> DVE chain
    sc_et = et.tile([128, 2, T], F32, name="sc", tag="sc")
    for e in (0, 1):
        for c in range(CH):
            nc.scalar.activation(sc_et[:, e, c * CHT:(c + 1) * CHT],
                                 accs[e][c], AF.Sigmoid, scale=1.0 / SCALE)
    for t4 in range(T // 128):
        tsl = slice(t4 * 128, (t4 + 1) * 128)
        ot = pot.tile([128, 256], F32, name=f"ot_{t4}", tag="ot")
        for e in (0, 1):
            nc.tensor.transpose(ot[:, e * 128:(e + 1) * 128],
                                sc_et[:, e, tsl], ident)
        # scores[t, e] PSUM -> SBUF on ACT so the 4 pot banks recycle fast
        cp = tkp.tile([128, 256], F32, name=f"cp_{t4}", tag="cp")
        nc.scalar.copy(cp, ot)
        swb = tkp.tile([128, 256], F32, name=f"swb_{t4}", tag="swb")
        nc.vector.tensor_tensor(swb, cp, bias_rep, op=OP.add)
        swb_g = swb.rearrange("p (g s) -> p g s", s=32)

        m1 = tkp.tile([128, 8], F32, name=f"m1_{t4}", tag="m1")
        nc.vector.tensor_reduce(m1, swb_g, axis=AX.X, op=OP.max)
        swb2 = tkp.tile([128, 256], F32, name=f"swb2_{t4}", tag="swb2")
        nc.vector.match_replace(out=swb2, in_to_replace=m1,
                                in_values=swb, imm_value=-1e30)
        gsum = tkp.tile([128, 8], F32, name=f"gsum_{t4}", tag="gsum")
        nc.vector.tensor_reduce(gsum,
                                swb2.rearrange("p (g s) -> p g s", s=32),
                                axis=AX.X, op=OP.max)
        nc.vector.tensor_tensor(gsum, gsum, m1, op=OP.add)
        g8 = tkp.tile([128, 8], F32, name=f"g8_{t4}", tag="g8")
        nc.vector.max(out=g8, in_=gsum)
        # swbm = (group kept ? swb : 0) in one fused op
        swbm = tkp.tile([128, 256], F32, name=f"swbm_{t4}", tag="swbm")
        nc.vector.scalar_tensor_tensor(
            swbm.rearrange("p (g s) -> p g s", s=32),
            gsum[:, :, None].to_broadcast([128, 8, 32]), g8[:, 3:4],
            swb_g, op0=OP.is_ge, op1=OP.mult)
        top8 = tkp.tile([128, 8], F32, name=f"top8_{t4}", tag="top8")
        nc.vector.max(out=top8, in_=swbm)
        # sel = (swbm >= top8[7]) * scores, with row-sum accumulator
        sel = tkp.tile([128, 256], F32, name=f"sel_{t4}", tag="sel")
        ssum = tkp.tile([128, 1], F32, name=f"ssum_{t4}", tag="ssum")
        nc.vector.scalar_tensor_tensor(sel, swbm, top8[:, 7:8], cp,
                                       op0=OP.is_ge, op1=OP.mult,
                                       accum_out=ssum)
        inv = tkp.tile([128, 1], F32, name=f"inv_{t4}", tag="inv")
        nc.vector.reciprocal(inv, ssum)
        ow = tkp.tile([128, 256], F16, name=f"ow_{t4}", tag="ow")
        nc.vector.tensor_scalar(ow, sel, inv, ROUTED_SCALING,
                                op0=OP.mult, op1=OP.mult)
        nc.sync.dma_start(out_d[tsl, :], ow)


def _build():
    nc = bacc.Bacc("TRN2", target_bir_lowering=False, debug=False)
    aps = _declare_io(nc)
    with tile.TileContext(nc) as tc:
        with contextlib.ExitStack() as ctx:
            pools = _make_pools(tc, ctx)
            _body(nc, pools, *aps)
    nc.compile()
    return nc


def _lay_x(a, dtype):
    # [1024, 7168] -> [128hp, 56k, 1024t], contiguous
    return np.ascontiguousarray(
        a.reshape(T, KT, 128).transpose(2, 1, 0).astype(dtype, copy=False))


def _lay_w(a, dtype):
    # [256, 7168] -> [128hp, 56k, 256e], contiguous
    return np.ascontiguousarray(
        a.T.reshape(KT, 128, E).transpose(1, 0, 2).astype(dtype, copy=False))


def _make_in_maps(hidden_states, weight, e_score_correction_bias):
    f32 = np.float32
    f8 = mybir.dt.np(F8)
    x = np.asarray(hidden_states, f32)
    w = np.asarray(weight, f32)
    b = np.asarray(e_score_correction_bias, f32)

    w16 = w.astype(np.float16)
    w16s = (w16.astype(f32) * SCALE).astype(np.float16)
    wl8 = ((w - w16.astype(f32)) * SCALE).astype(f8)
    w8 = (w16.astype(f32) * 8.0).astype(f8)
    w16s_l = _lay_w(w16s, np.float16)
    wl8_l = _lay_w(wl8, f8)
    w8_l = _lay_w(w8, f8)
    ident_np = np.eye(128, dtype=f32)

    in_maps = []
    for i in range(N_CORES):
        xc = x[i * T:(i + 1) * T]
        x16 = xc.astype(np.float16)
        xl8 = ((xc - x16.astype(f32)) * S_XL).astype(f8)
        in_maps.append({
            "x16": _lay_x(x16, np.float16),
            "xl8": _lay_x(xl8, f8),
            "w16s": w16s_l, "wl8": wl8_l, "w8": w8_l,
            "bias_plain": b, "ident": ident_np,
        })
    return in_maps


def kernel(hidden_states, weight, e_score_correction_bias):
    in_maps = _make_in_maps(hidden_states, weight, e_score_correction_bias)
    if "nc" not in _CACHE:
        _CACHE["nc"] = _build()
    nc = _CACHE["nc"]
    res = bass_utils.run_bass_kernel_spmd(nc, in_maps, core_ids=list(range(N_CORES)))
    return np.concatenate(
        [res.results[i]["out"].astype(np.float32) for i in range(N_CORES)], axis=0)


if __name__ == "__main__":
    rng = np.random.default_rng(0)
    hs = rng.standard_normal((TOKENS, HIDDEN)).astype(np.float32)
    w = (rng.standard_normal((E, HIDDEN)) * 0.02).astype(np.float32)
    b = (rng.standard_normal(E) * 0.1).astype(np.float32)
    out = kernel(hs, w, b)
    print(out.shape, out.dtype, np.isfinite(out).all())


# revision 12
# speedup vs baseline: 1.1097x; 1.1097x over previous
"""DeepseekV3 MoE gate kernel for 8 TRN2 NeuronCores — v2.

Strategy: shard tokens 8192 -> 8 x 1024, replicate gate weight/bias.

GEMM (per core, logits^T[e,t] accumulated in PSUM at 2^16 scale):
  - pass 1 (fp16): w16s = fp16(w) * 2^16 stationary, x16 = fp16(x) moving.
  - pass 2 (fp8 DoubleRow): acc += w8^T xl8 + wl8^T x8 (both fp16-residual
    correction terms in one DR matmul per (k, e-tile, chunk)).
  v2 changes vs the 173.7us baseline:
  - x8 = e4m3(x16) is no longer shipped from host: the otherwise-idle ACT
    engine derives it on-chip (Copy is a filler function present in every
    ACT table set, so it never evicts the Sigmoid table). DMA drops from
    36.7 -> 29.4 MB/core.
  - k-outer / chunk-inner matmul loops: each stationary weight tile (fp16
    and the expensive 256-col DoubleRow pair) is loaded once and serves
    both 512-token chunks back-to-back, hiding LDWEIGHTS under the longer
    matmul stream.
  - epilogue: all 4 sigmoids run first so the PSUM accumulators free
    early (pacc bufs=1 + pot bufs=4 exactly fills the 8 PSUM banks); ACT
    copies each transposed score tile PSUM->SBUF so the PE transpose
    buffers recycle fast; the top-k chain then runs DVE-only as before.
"""
import contextlib
import sys

for _p in ("/opt/trn_rl_repo", "/opt/pypackages"):
    if _p not in sys.path:
        sys.path.append(_p)

import numpy as np
import concourse.bacc as bacc
import concourse.mybir as mybir
import concourse.tile as tile
from concourse import bass_utils

F32 = mybir.dt.float32
F16 = mybir.dt.float16
F8 = mybir.dt.float8e4
DR = mybir.MatmulPerfMode.DoubleRow
AF = mybir.ActivationFunctionType
OP = mybir.AluOpType
AX = mybir.AxisListType

TOKENS, HIDDEN, E = 8192, 7168, 256
N_CORES = 8
T = TOKENS // N_CORES          # 1024 tokens per core
KT = HIDDEN // 128             # 56 k-tiles
CH = 2                         # chunks (PSUM bank limit: 512 fp32/matmul)
CHT = T // CH                  # 512 tokens per chunk
GK = 8                         # k-tiles per streamed group
NG = KT // GK                  # 7 groups
SCALE = 2.0 ** 16              # pass-1/2 common scale
S_XL = 2.0 ** 13               # xl8 = e4m3(xl * S_XL); w8 = e4m3(w * 8)
ROUTED_SCALING = 2.5

_CACHE = {}


def _declare_io(nc):
    x16_d = nc.dram_tensor("x16", [128, KT, T], F16, kind="ExternalInput").ap()
    xl8_d = nc.dram_tensor("xl8", [128, KT, T], F8, kind="ExternalInput").ap()
    w16_d = nc.dram_tensor("w16s", [128, KT, E], F16, kind="ExternalInput").ap()
    wl8_d = nc.dram_tensor("wl8", [128, KT, E], F8, kind="ExternalInput").ap()
    w8_d = nc.dram_tensor("w8", [128, KT, E], F8, kind="ExternalInput").ap()
    biasp_d = nc.dram_tensor("bias_plain", [E], F32, kind="ExternalInput").ap()
    ident_d = nc.dram_tensor("ident", [128, 128], F32, kind="ExternalInput").ap()
    out_d = nc.dram_tensor("out", [T, E], F16, kind="ExternalOutput").ap()
    return x16_d, xl8_d, w16_d, wl8_d, w8_d, biasp_d, ident_d, out_d


def _make_pools(tc, ctx):
    return {
        "const": ctx.enter_context(tc.tile_pool(name="const", bufs=1)),
        "wt": ctx.enter_context(tc.tile_pool(name="wt", bufs=1)),
        "xf": ctx.enter_context(tc.tile_pool(name="xf", bufs=3)),
        "xq": ctx.enter_context(tc.tile_pool(name="xq", bufs=3)),
        "et": ctx.enter_context(tc.tile_pool(name="et", bufs=2)),
        "tk": ctx.enter_context(tc.tile_pool(name="tk", bufs=2)),
        "cp": ctx.enter_context(tc.tile_pool(name="cp", bufs=2)),
        "pacc": ctx.enter_context(tc.tile_pool(name="pacc", bufs=2, space="PSUM")),
        "pot": ctx.enter_context(tc.tile_pool(name="pot", bufs=4, space="PSUM")),
    }


def _body(nc, pools, x16_d, xl8_d, w16_d, wl8_d, w8_d, biasp_d, ident_d, out_d,
          tc=None, gemm_only=False):
    """Kernel body, chunk-outer pipelined: chunk 0's epilogue overlaps
    chunk 1's GEMM. If `tc` is given (timing loop), staggered-reset stage
    boundaries land at chunk-aligned points so iteration i+1's GEMM can
    overlap iteration i's final top-k chain."""
    const, xf, xq = pools["const"], pools["xf"], pools["xq"]
    wt = pools["wt"]
    et, tkp = pools["et"], pools["tk"]
    cpp = pools["cp"]
    pacc, pot = pools["pacc"], pools["pot"]

    w16s_t = wt.tile([128, KT, E], F16, name="w16s_t", tag="w16s")
    w8p_t = wt.tile([128, 2, KT, E], F8, name="w8p_t", tag="w8p")
    bias_rep = const.tile([128, E], F32, name="bias_rep")
    ident = const.tile([128, 128], F32, name="ident")
    nc.sync.dma_start(ident, ident_d)

    def stream_weights(g):
        ks = slice(g * GK, (g + 1) * GK)
        nc.sync.dma_start(w16s_t[:, ks], w16_d[:, ks])
        nc.sync.dma_start(w8p_t[:, 1, ks], wl8_d[:, ks])
        nc.sync.dma_start(w8p_t[:, 0, ks], w8_d[:, ks])

    def gemm_chunk(c):
        csl = slice(c * CHT, (c + 1) * CHT)
        accs = [pacc.tile([128, CHT], F32, name=f"acc{e}_{c}", tag=f"acc{e}")
                for e in (0, 1)]
        xqgs = []

        def dr_mm(gd, ko, e):
            k = gd * GK + ko
            nc.tensor.matmul(
                accs[e], w8p_t[:, :, k, e * 128:(e + 1) * 128],
                xqgs[gd][:, :, ko], start=False, stop=(k == KT - 1),
                perf_mode=DR)

        for g in range(NG):
            gsl = slice(g * GK, (g + 1) * GK)
            x16g = xf.tile([128, GK, CHT], F16, name=f"x16_{c}_{g}", tag="x16")
            src = x16_d[:, gsl, csl]
            if c == 0 and g == 0:
                stream_weights(0)
                # halve the very first transfer so the first matmuls start
                # as soon as the leading k-tiles land
                nc.sync.dma_start(x16g[:, 0:GK // 2], src[:, 0:GK // 2])
                nc.sync.dma_start(x16g[:, GK // 2:], src[:, GK // 2:])
            else:
                nc.sync.dma_start(x16g, src)
            xqg = xq.tile([128, 2, GK, CHT], F8, name=f"xq_{c}_{g}", tag="xq")
            xqgs.append(xqg)
            nc.sync.dma_start(xqg[:, 0], xl8_d[:, gsl, csl])
            # derive x8 = e4m3(x16) on ACT (Copy lives in every table set)
            nc.scalar.copy(xqg[:, 1], x16g)
            if c == 0 and g < NG - 1:
                stream_weights(g + 1)
            if c == 0 and g == 0:
                nc.sync.dma_start(bias_rep,
                                  biasp_d[None, :].to_broadcast([128, E]))
            for ko in range(GK):
                k = g * GK + ko
                for e in (0, 1):
                    nc.tensor.matmul(
                        accs[e], w16s_t[:, k, e * 128:(e + 1) * 128],
                        x16g[:, ko], start=(k == 0), stop=False)
            for ko in range(GK):
                for e in (0, 1):
                    dr_mm(g, ko, e)
        return accs

    def epi_head(c, accs):
        """sigmoids (frees PSUM accs) + PE transposes + ACT copies."""
        sc = et.tile([128, 2, CHT], F32, name=f"sc_{c}", tag="sc")
        for e in (0, 1):
            nc.scalar.activation(sc[:, e], accs[e], AF.Sigmoid,
                                 scale=1.0 / SCALE)
        cp_a = cpp.tile([128, 4, 256], F32, name=f"cp_{c}", tag="cp")
        for t4 in range(4):
            tsl = slice(t4 * 128, (t4 + 1) * 128)
            ot = pot.tile([128, 256], F32, name=f"ot_{c}_{t4}", tag="ot")
            for e in (0, 1):
                nc.tensor.transpose(ot[:, e * 128:(e + 1) * 128],
                                    sc[:, e, tsl], ident)
            nc.scalar.copy(cp_a[:, t4], ot)
        return cp_a

    def epi_chain(c, cp_a):
        """top-k chain, DVE-only; batchable ops widened over the 4
        128-token groups of the chunk."""
        bias4 = bias_rep[:, None, :].to_broadcast([128, 4, 256])
        swb = tkp.tile([128, 4, 256], F32, name=f"swb_{c}", tag="swb")
        nc.vector.scalar_tensor_tensor(swb, cp_a, 0.0, bias4,
                                       op0=OP.add, op1=OP.add)
        swb_g = swb.rearrange("p f (g s) -> p f g s", s=32)
        m1 = tkp.tile([128, 4, 8], F32, name=f"m1_{c}", tag="m1")
        nc.vector.tensor_reduce(m1, swb_g, axis=AX.X, op=OP.max)
        swb2 = tkp.tile([128, 4, 256], F32, name=f"swb2_{c}", tag="swb2")
        for t4 in range(4):
            nc.vector.match_replace(out=swb2[:, t4], in_to_replace=m1[:, t4],
                                    in_values=swb[:, t4], imm_value=-1e30)
        gsum = tkp.tile([128, 4, 8], F32, name=f"gsum_{c}", tag="gsum")
        nc.vector.tensor_reduce(gsum,
                                swb2.rearrange("p f (g s) -> p f g s", s=32),
                                axis=AX.X, op=OP.max)
        nc.vector.tensor_tensor(gsum, gsum, m1, op=OP.add)
        g8 = tkp.tile([128, 4, 8], F32, name=f"g8_{c}", tag="g8")
        for t4 in range(4):
            nc.vector.max(out=g8[:, t4], in_=gsum[:, t4])
        # gm = (group kept?), then swbm = gm * swb — both batched
        gm = tkp.tile([128, 4, 8], F32, name=f"gm_{c}", tag="gm")
        nc.vector.tensor_tensor(gm, gsum,
                                g8[:, :, 3:4].to_broadcast([128, 4, 8]),
                                op=OP.is_ge)
        swbm = tkp.tile([128, 4, 256], F32, name=f"swbm_{c}", tag="swbm")
        nc.vector.scalar_tensor_tensor(
            swbm.rearrange("p f (g s) -> p f g s", s=32),
            gm[:, :, :, None].to_broadcast([128, 4, 8, 32]), 0.0,
            swb_g, op0=OP.add, op1=OP.mult)
        top8 = tkp.tile([128, 4, 8], F32, name=f"top8_{c}", tag="top8")
        sel = tkp.tile([128, 4, 256], F32, name=f"sel_{c}", tag="sel")
        ssum = tkp.tile([128, 4], F32, name=f"ssum_{c}", tag="ssum")
        for t4 in range(4):
            nc.vector.max(out=top8[:, t4], in_=swbm[:, t4])
            nc.vector.scalar_tensor_tensor(sel[:, t4], swbm[:, t4],
                                           top8[:, t4, 7:8], cp_a[:, t4],
                                           op0=OP.is_ge, op1=OP.mult,
                                           accum_out=ssum[:, t4:t4 + 1])
        inv = tkp.tile([128, 4], F32, name=f"inv_{c}", tag="inv")
        nc.vector.reciprocal(inv, ssum)
        ow = tkp.tile([128, 4, 256], F16, name=f"ow_{c}", tag="ow")
        nc.vector.scalar_tensor_tensor(
            ow, sel, ROUTED_SCALING,
            inv[:, :, None].to_broadcast([128, 4, 256]),
            op0=OP.mult, op1=OP.mult)
        # one output DMA per chunk, via GPSIMD/SWDGE: on the sync HWDGE
        # ring it would queue ahead of later input DMAs and stall them
        # until the chain finishes (the sequencer blocks on the sem)
        dst = out_d[c * CHT:(c + 1) * CHT, :].rearrange(
            "(f p) e -> p f e", p=128)
        nc.gpsimd.dma_start(dst, ow)

    accs0 = gemm_chunk(0)
    if tc is not None:
        tc.stage_boundary()
    cp0 = epi_head(0, accs0)
    epi_chain(0, cp0)
    accs1 = gemm_chunk(1)
    if gemm_only:
        if tc is not None:
            tc.stage_boundary()
            tc.stage_boundary()
        for e in (0, 1):
            sc = et.tile([128, CHT], F32, name=f"drain{e}", tag=f"drain{e}")
            nc.scalar.activation(sc, accs1[e], AF.Sigmoid, scale=1.0 / SCALE)
        return
    if tc is not None:
        tc.stage_boundary()
    cp1 = epi_head(1, accs1)
    if tc is not None:
        tc.stage_boundary()
    epi_chain(1, cp1)


def _build():
    nc = bacc.Bacc("TRN2", target_bir_lowering=False, debug=False)
    aps = _declare_io(nc)
    with tile.TileContext(nc) as tc:
        with contextlib.ExitStack() as ctx:
            pools = _make_pools(tc, ctx)
            _body(nc, pools, *aps)
    nc.compile()
    return nc


def _lay_x(a, dtype):
    # [1024, 7168] -> [128hp, 56k, 1024t], contiguous
    return np.ascontiguousarray(
        a.reshape(T, KT, 128).transpose(2, 1, 0).astype(dtype, copy=False))


def _lay_w(a, dtype):
    # [256, 7168] -> [128hp, 56k, 256e], contiguous
    return np.ascontiguousarray(
        a.T.reshape(KT, 128, E).transpose(1, 0, 2).astype(dtype, copy=False))


def _make_in_maps(hidden_states, weight, e_score_correction_bias):
    f32 = np.float32
    f8 = mybir.dt.np(F8)
    x = np.asarray(hidden_states, f32)
    w = np.asarray(weight, f32)
    b = np.asarray(e_score_correction_bias, f32)

    w16 = w.astype(np.float16)
    w16s = (w16.astype(f32) * SCALE).astype(np.float16)
    wl8 = ((w - w16.astype(f32)) * SCALE).astype(f8)
    w8 = (w16.astype(f32) * 8.0).astype(f8)
    w16s_l = _lay_w(w16s, np.float16)
    wl8_l = _lay_w(wl8, f8)
    w8_l = _lay_w(w8, f8)
    ident_np = np.eye(128, dtype=f32)

    in_maps = []
    for i in range(N_CORES):
        xc = x[i * T:(i + 1) * T]
        x16 = xc.astype(np.float16)
        xl8 = ((xc - x16.astype(f32)) * S_XL).astype(f8)
        in_maps.append({
            "x16": _lay_x(x16, np.float16),
            "xl8": _lay_x(xl8, f8),
            "w16s": w16s_l, "wl8": wl8_l, "w8": w8_l,
            "bias_plain": b, "ident": ident_np,
        })
    return in_maps


def kernel(hidden_states, weight, e_score_correction_bias):
    in_maps = _make_in_maps(hidden_states, weight, e_score_correction_bias)
    if "nc" not in _CACHE:
        _CACHE["nc"] = _build()
    nc = _CACHE["nc"]
    res = bass_utils.run_bass_kernel_spmd(nc, in_maps, core_ids=list(range(N_CORES)))
    return np.concatenate(
        [res.results[i]["out"].astype(np.float32) for i in range(N_CORES)], axis=0)


if __name__ == "__main__":
    rng = np.random.default_rng(0)
    hs = rng.standard_normal((TOKENS, HIDDEN)).astype(np.float32)
    w = (rng.standard_normal((E, HIDDEN)) * 0.02).astype(np.float32)
    b = (rng.standard_normal(E) * 0.1).astype(np.float32)
    out = kernel(hs, w, b)
    print(out.shape, out.dtype, np.isfinite(out).all())


# revision 16
# speedup vs baseline: 1.3205x; 1.1899x over previous
"""DeepseekV3 MoE gate kernel for 8 TRN2 NeuronCores — v2.

Strategy: shard tokens 8192 -> 8 x 1024, replicate gate weight/bias.

GEMM (per core, logits^T[e,t] accumulated in PSUM at 2^16 scale):
  - pass 1 (fp16): w16s = fp16(w) * 2^16 stationary, x16 = fp16(x) moving.
  - pass 2 (fp8 DoubleRow): acc += w8^T xl8 + wl8^T x8 (both fp16-residual
    correction terms in one DR matmul per (k, e-tile, chunk)).
  v2 changes vs the 173.7us baseline:
  - x8 = e4m3(x16) is no longer shipped from host: the otherwise-idle ACT
    engine derives it on-chip (Copy is a filler function present in every
    ACT table set, so it never evicts the Sigmoid table). DMA drops from
    36.7 -> 29.4 MB/core.
  - k-outer / chunk-inner matmul loops: each stationary weight tile (fp16
    and the expensive 256-col DoubleRow pair) is loaded once and serves
    both 512-token chunks back-to-back, hiding LDWEIGHTS under the longer
    matmul stream.
  - epilogue: all 4 sigmoids run first so the PSUM accumulators free
    early (pacc bufs=1 + pot bufs=4 exactly fills the 8 PSUM banks); ACT
    copies each transposed score tile PSUM->SBUF so the PE transpose
    buffers recycle fast; the top-k chain then runs DVE-only as before.
"""
import contextlib
import sys

for _p in ("/opt/trn_rl_repo", "/opt/pypackages"):
    if _p not in sys.path:
        sys.path.append(_p)

import numpy as np
import concourse.bacc as bacc
import concourse.mybir as mybir
import concourse.tile as tile
from concourse import bass_utils

F32 = mybir.dt.float32
F16 = mybir.dt.float16
F8 = mybir.dt.float8e4
DR = mybir.MatmulPerfMode.DoubleRow
AF = mybir.ActivationFunctionType
OP = mybir.AluOpType
AX = mybir.AxisListType

TOKENS, HIDDEN, E = 8192, 7168, 256
N_CORES = 8
T = TOKENS // N_CORES          # 1024 tokens per core
KT = HIDDEN // 128             # 56 k-tiles
CH = 2                         # chunks (PSUM bank limit: 512 fp32/matmul)
CHT = T // CH                  # 512 tokens per chunk
GK = 8                         # k-tiles per streamed group
NG = KT // GK                  # 7 groups
SCALE = 2.0 ** 16              # pass-1/2 common scale
S_XL = 2.0 ** 13               # xl8 = e4m3(xl * S_XL); w8 = e4m3(w * 8)
ROUTED_SCALING = 2.5

_CACHE = {}

# experiment knobs (diag flips these; safe defaults)
XQ_ADJ = False      # xq slot-adjacent layout for the DR moving operand
ONE_SIDED = False   # pass 2 corrects only the x residual (plain fp8, no DR)


def _declare_io(nc):
    x16_d = nc.dram_tensor("x16", [128, KT, T], F16, kind="ExternalInput").ap()
    xl8_d = nc.dram_tensor("xl8", [128, KT, T], F8, kind="ExternalInput").ap()
    w16_d = nc.dram_tensor("w16s", [128, KT, E], F16, kind="ExternalInput").ap()
    wl8_d = nc.dram_tensor("wl8", [128, KT, E], F8, kind="ExternalInput").ap()
    w8_d = nc.dram_tensor("w8", [128, KT, E], F8, kind="ExternalInput").ap()
    biasp_d = nc.dram_tensor("bias_plain", [E], F32, kind="ExternalInput").ap()
    ident_d = nc.dram_tensor("ident", [128, 128], F32, kind="ExternalInput").ap()
    out_d = nc.dram_tensor("out", [T, E], F16, kind="ExternalOutput").ap()
    return x16_d, xl8_d, w16_d, wl8_d, w8_d, biasp_d, ident_d, out_d


def _make_pools(tc, ctx):
    return {
        "const": ctx.enter_context(tc.tile_pool(name="const", bufs=1)),
        "wt": ctx.enter_context(tc.tile_pool(name="wt", bufs=1)),
        "xf": ctx.enter_context(tc.tile_pool(name="xf", bufs=3)),
        "xq": ctx.enter_context(tc.tile_pool(name="xq", bufs=3)),
        "et": ctx.enter_context(tc.tile_pool(name="et", bufs=2)),
        "tk": ctx.enter_context(tc.tile_pool(name="tk", bufs=2)),
        "cp": ctx.enter_context(tc.tile_pool(name="cp", bufs=2)),
        "pacc": ctx.enter_context(tc.tile_pool(name="pacc", bufs=2, space="PSUM")),
        "pot": ctx.enter_context(tc.tile_pool(name="pot", bufs=4, space="PSUM")),
    }


def _body(nc, pools, x16_d, xl8_d, w16_d, wl8_d, w8_d, biasp_d, ident_d, out_d,
          tc=None, gemm_only=False):
    """Kernel body, chunk-outer pipelined: chunk 0's epilogue overlaps
    chunk 1's GEMM. If `tc` is given (timing loop), staggered-reset stage
    boundaries land at chunk-aligned points so iteration i+1's GEMM can
    overlap iteration i's final top-k chain."""
    const, xf, xq = pools["const"], pools["xf"], pools["xq"]
    wt = pools["wt"]
    et, tkp = pools["et"], pools["tk"]
    cpp = pools["cp"]
    pacc, pot = pools["pacc"], pools["pot"]

    w16s_t = wt.tile([128, KT, E], F16, name="w16s_t", tag="w16s")
    w8p_t = wt.tile([128, 2, KT, E], F8, name="w8p_t", tag="w8p")
    bias_rep = const.tile([128, E], F32, name="bias_rep")
    ident = const.tile([128, 128], F32, name="ident")
    nc.sync.dma_start(ident, ident_d)

    def stream_weights(g):
        ks = slice(g * GK, (g + 1) * GK)
        nc.sync.dma_start(w16s_t[:, ks], w16_d[:, ks])
        if not ONE_SIDED:
            nc.sync.dma_start(w8p_t[:, 1, ks], wl8_d[:, ks])
        nc.sync.dma_start(w8p_t[:, 0, ks], w8_d[:, ks])

    def gemm_chunk(c):
        csl = slice(c * CHT, (c + 1) * CHT)
        accs = [pacc.tile([128, CHT], F32, name=f"acc{e}_{c}", tag=f"acc{e}")
                for e in (0, 1)]
        xqgs = []

        def dr_mm(gd, ko, e):
            k = gd * GK + ko
            es = slice(e * 128, (e + 1) * 128)
            xqg = xqgs[gd]
            rhs = xqg[:, ko, :, :] if XQ_ADJ else xqg[:, :, ko]
            if ONE_SIDED:
                nc.tensor.matmul(
                    accs[e], w8p_t[:, 0, k, es], rhs[:, 0],
                    start=False, stop=(k == KT - 1))
            else:
                nc.tensor.matmul(
                    accs[e], w8p_t[:, :, k, es], rhs,
                    start=False, stop=(k == KT - 1), perf_mode=DR)

        for g in range(NG):
            gsl = slice(g * GK, (g + 1) * GK)
            x16g = xf.tile([128, GK, CHT], F16, name=f"x16_{c}_{g}", tag="x16")
            src = x16_d[:, gsl, csl]
            if c == 0 and g == 0:
                stream_weights(0)
                # halve the very first transfer so the first matmuls start
                # as soon as the leading k-tiles land
                nc.sync.dma_start(x16g[:, 0:GK // 2], src[:, 0:GK // 2])
                nc.sync.dma_start(x16g[:, GK // 2:], src[:, GK // 2:])
            else:
                nc.sync.dma_start(x16g, src)
            shape = [128, GK, 2, CHT] if XQ_ADJ else [128, 2, GK, CHT]
            xqg = xq.tile(shape, F8, name=f"xq_{c}_{g}", tag="xq")
            xqgs.append(xqg)
            xl_dst = xqg[:, :, 0] if XQ_ADJ else xqg[:, 0]
            nc.sync.dma_start(xl_dst, xl8_d[:, gsl, csl])
            if not ONE_SIDED:
                # derive x8 = e4m3(x16) on ACT (Copy is in every table set)
                x8_dst = xqg[:, :, 1] if XQ_ADJ else xqg[:, 1]
                nc.scalar.copy(x8_dst, x16g)
            if c == 0 and g < NG - 1:
                stream_weights(g + 1)
            if c == 0 and g == 0:
                nc.sync.dma_start(bias_rep,
                                  biasp_d[None, :].to_broadcast([128, E]))
            for ko in range(GK):
                k = g * GK + ko
                for e in (0, 1):
                    nc.tensor.matmul(
                        accs[e], w16s_t[:, k, e * 128:(e + 1) * 128],
                        x16g[:, ko], start=(k == 0), stop=False)
            for ko in range(GK):
                for e in (0, 1):
                    dr_mm(g, ko, e)
        return accs

    def epi_head(c, accs):
        """sigmoids (frees PSUM accs) + PE transposes + ACT copies."""
        sc = et.tile([128, 2, CHT], F32, name=f"sc_{c}", tag="sc")
        for e in (0, 1):
            nc.scalar.activation(sc[:, e], accs[e], AF.Sigmoid,
                                 scale=1.0 / SCALE)
        cp_a = cpp.tile([128, 4, 256], F32, name=f"cp_{c}", tag="cp")
        for t4 in range(4):
            tsl = slice(t4 * 128, (t4 + 1) * 128)
            ot = pot.tile([128, 256], F32, name=f"ot_{c}_{t4}", tag="ot")
            for e in (0, 1):
                nc.tensor.transpose(ot[:, e * 128:(e + 1) * 128],
                                    sc[:, e, tsl], ident)
            nc.scalar.copy(cp_a[:, t4], ot)
        return cp_a

    def epi_chain(c, cp_a):
        """top-k chain, DVE-only; batchable ops widened over the 4
        128-token groups of the chunk."""
        bias4 = bias_rep[:, None, :].to_broadcast([128, 4, 256])
        swb = tkp.tile([128, 4, 256], F32, name=f"swb_{c}", tag="swb")
        nc.vector.scalar_tensor_tensor(swb, cp_a, 0.0, bias4,
                                       op0=OP.add, op1=OP.add)
        swb_g = swb.rearrange("p f (g s) -> p f g s", s=32)
        m1 = tkp.tile([128, 4, 8], F32, name=f"m1_{c}", tag="m1")
        nc.vector.tensor_reduce(m1, swb_g, axis=AX.X, op=OP.max)
        swb2 = tkp.tile([128, 4, 256], F32, name=f"swb2_{c}", tag="swb2")
        for t4 in range(4):
            nc.vector.match_replace(out=swb2[:, t4], in_to_replace=m1[:, t4],
                                    in_values=swb[:, t4], imm_value=-1e30)
        gsum = tkp.tile([128, 4, 8], F32, name=f"gsum_{c}", tag="gsum")
        nc.vector.tensor_reduce(gsum,
                                swb2.rearrange("p f (g s) -> p f g s", s=32),
                                axis=AX.X, op=OP.max)
        nc.vector.tensor_tensor(gsum, gsum, m1, op=OP.add)
        g8 = tkp.tile([128, 4, 8], F32, name=f"g8_{c}", tag="g8")
        for t4 in range(4):
            nc.vector.max(out=g8[:, t4], in_=gsum[:, t4])
        # gm = (group kept?), then swbm = gm * swb — both batched
        gm = tkp.tile([128, 4, 8], F32, name=f"gm_{c}", tag="gm")
        nc.vector.tensor_tensor(gm, gsum,
                                g8[:, :, 3:4].to_broadcast([128, 4, 8]),
                                op=OP.is_ge)
        swbm = tkp.tile([128, 4, 256], F32, name=f"swbm_{c}", tag="swbm")
        nc.vector.scalar_tensor_tensor(
            swbm.rearrange("p f (g s) -> p f g s", s=32),
            gm[:, :, :, None].to_broadcast([128, 4, 8, 32]), 0.0,
            swb_g, op0=OP.add, op1=OP.mult)
        top8 = tkp.tile([128, 4, 8], F32, name=f"top8_{c}", tag="top8")
        sel = tkp.tile([128, 4, 256], F32, name=f"sel_{c}", tag="sel")
        ssum = tkp.tile([128, 4], F32, name=f"ssum_{c}", tag="ssum")
        for t4 in range(4):
            nc.vector.max(out=top8[:, t4], in_=swbm[:, t4])
            nc.vector.scalar_tensor_tensor(sel[:, t4], swbm[:, t4],
                                           top8[:, t4, 7:8], cp_a[:, t4],
                                           op0=OP.is_ge, op1=OP.mult,
                                           accum_out=ssum[:, t4:t4 + 1])
        inv = tkp.tile([128, 4], F32, name=f"inv_{c}", tag="inv")
        nc.vector.reciprocal(inv, ssum)
        ow = tkp.tile([128, 4, 256], F16, name=f"ow_{c}", tag="ow")
        nc.vector.scalar_tensor_tensor(
            ow, sel, ROUTED_SCALING,
            inv[:, :, None].to_broadcast([128, 4, 256]),
            op0=OP.mult, op1=OP.mult)
        # one output DMA per chunk, via GPSIMD/SWDGE: on the sync HWDGE
        # ring it would queue ahead of later input DMAs and stall them
        # until the chain finishes (the sequencer blocks on the sem)
        dst = out_d[c * CHT:(c + 1) * CHT, :].rearrange(
            "(f p) e -> p f e", p=128)
        nc.gpsimd.dma_start(dst, ow)

    accs0 = gemm_chunk(0)
    if tc is not None:
        tc.stage_boundary()
    cp0 = epi_head(0, accs0)
    epi_chain(0, cp0)
    accs1 = gemm_chunk(1)
    if gemm_only:
        if tc is not None:
            tc.stage_boundary()
            tc.stage_boundary()
        for e in (0, 1):
            sc = et.tile([128, CHT], F32, name=f"drain{e}", tag=f"drain{e}")
            nc.scalar.activation(sc, accs1[e], AF.Sigmoid, scale=1.0 / SCALE)
        return
    if tc is not None:
        tc.stage_boundary()
    cp1 = epi_head(1, accs1)
    if tc is not None:
        tc.stage_boundary()
    epi_chain(1, cp1)


def _build():
    nc = bacc.Bacc("TRN2", target_bir_lowering=False, debug=False)
    aps = _declare_io(nc)
    with tile.TileContext(nc) as tc:
        with contextlib.ExitStack() as ctx:
            pools = _make_pools(tc, ctx)
            _body(nc, pools, *aps)
    nc.compile()
    return nc


def _lay_x(a, dtype):
    # [1024, 7168] -> [128hp, 56k, 1024t], contiguous
    return np.ascontiguousarray(
        a.reshape(T, KT, 128).transpose(2, 1, 0).astype(dtype, copy=False))


def _lay_w(a, dtype):
    # [256, 7168] -> [128hp, 56k, 256e], contiguous
    return np.ascontiguousarray(
        a.T.reshape(KT, 128, E).transpose(1, 0, 2).astype(dtype, copy=False))


def _make_in_maps(hidden_states, weight, e_score_correction_bias):
    f32 = np.float32
    f8 = mybir.dt.np(F8)
    x = np.asarray(hidden_states, f32)
    w = np.asarray(weight, f32)
    b = np.asarray(e_score_correction_bias, f32)

    w16 = w.astype(np.float16)
    w16s = (w16.astype(f32) * SCALE).astype(np.float16)
    wl8 = ((w - w16.astype(f32)) * SCALE).astype(f8)
    w8 = (w16.astype(f32) * 8.0).astype(f8)
    w16s_l = _lay_w(w16s, np.float16)
    wl8_l = _lay_w(wl8, f8)
    w8_l = _lay_w(w8, f8)
    ident_np = np.eye(128, dtype=f32)

    in_maps = []
    for i in range(N_CORES):
        xc = x[i * T:(i + 1) * T]
        x16 = xc.astype(np.float16)
        xl8 = ((xc - x16.astype(f32)) * S_XL).astype(f8)
        in_maps.append({
            "x16": _lay_x(x16, np.float16),
            "xl8": _lay_x(xl8, f8),
            "w16s": w16s_l, "wl8": wl8_l, "w8": w8_l,
            "bias_plain": b, "ident": ident_np,
        })
    return in_maps


def kernel(hidden_states, weight, e_score_correction_bias):
    in_maps = _make_in_maps(hidden_states, weight, e_score_correction_bias)
    if "nc" not in _CACHE:
        _CACHE["nc"] = _build()
    nc = _CACHE["nc"]
    res = bass_utils.run_bass_kernel_spmd(nc, in_maps, core_ids=list(range(N_CORES)))
    return np.concatenate(
        [res.results[i]["out"].astype(np.float32) for i in range(N_CORES)], axis=0)


if __name__ == "__main__":
    rng = np.random.default_rng(0)
    hs = rng.standard_normal((TOKENS, HIDDEN)).astype(np.float32)
    w = (rng.standard_normal((E, HIDDEN)) * 0.02).astype(np.float32)
    b = (rng.standard_normal(E) * 0.1).astype(np.float32)
    out = kernel(hs, w, b)
    print(out.shape, out.dtype, np.isfinite(out).all())
